# revision 8
# baseline (speedup 1.0000x reference)
"""Causal scaled-dot-product attention on 8 TRN2 NeuronCores.

Problem: B=8, Tq=Tk=2048, D=512, f32, causal + key-padding mask.
Sharding: batch-parallel — core i handles batch element i; no collectives.

Per-core algorithm (one batch element, all on one NeuronCore):
  * S^T[k, q] = sum_d KT_chunk^T @ QT per 128-wide k chunk (PE bf16,
    4 accumulating matmuls); the strictly-lower-triangular -1e30 causal
    tile is folded into the same PSUM accumulation as a 5th matmul
    (ident.T @ tri adds tri elementwise).
  * P^T = exp(S^T * 1/sqrt(D) + key_bias[k]) on ScalarE (key padding
    mask folds into the per-partition activation bias).
  * out[q,:] += P^T_chunk^T @ V_chunk and denom[q] += P^T_chunk^T @ ones
    (PE, stationary reuse); PV emitted one chunk behind S^T to hide exp.
  * Per q-block epilogue: out *= 1/denom, stores on the gpsimd queue.

Data movement (the interesting part):
  * 12 warm-up matmuls on memset data run first: ~5us of continuous PE
    activity flips the HAM clock-gate to 8/8 (2.4 GHz) at ~3.5us instead
    of ~30us; all real matmuls then run warm.
  * K (groups 1-3) and V (all) are loaded by gpsimd software-DGE
    casting DMAs (f32 DRAM -> bf16 SBUF, conversion in the DMA engine,
    ~340 GB/s source-side, 1 MB per issue). V tiles are used directly.
  * K0 and Q stream as f32 on the HWDGE queues; Q is cast on DVE.
  * d-major (transposed) layouts come from XBAR transpose-DMAs
    (dma_start_transpose, bf16 SBUF->SBUF) instead of PE matmuls for
    K0-3 and Q2-3; Q0/Q1 are transposed on the PE (they are needed
    before the XBAR+cast chain could deliver them, and the PE is
    otherwise idle that early). XBAR destinations must be contiguous
    tiles (strided dsts silently corrupt on HW), hence the
    [128, tb, dc, 128] group layout with 2-free-dim matmul operands.
  * Output stores ride the gpsimd queue (idle after input casting),
    except the last group's odd q-blocks which use sync to halve the
    store tail.

No max-subtraction: post-scale scores are ~N(0,1), so exp is safe in
f32 and softmax is shift-invariant.
"""

import os

import numpy as np

B = 8
T = 2048
D = 512
P = 128
NEG = -1e30
SCALE = 1.0 / float(np.sqrt(np.float32(D)))

N_DSUB = D // P  # 4 d-chunks of 128
N_KCHUNK = T // P  # 16 k-chunks of 128
QGROUP = 512
N_GROUP = T // QGROUP  # 4 q-groups
SUBS = QGROUP // P  # 4 q/k-subblocks of 128 per group

_CACHE = {}


def _build():
    import concourse.bass as bass  # noqa: F401
    import concourse.mybir as mybir
    import concourse.tile as tile
    from concourse import bacc
    from concourse.masks import make_identity, make_lower_triangular

    f32 = mybir.dt.float32
    bf16 = mybir.dt.bfloat16
    i32 = mybir.dt.int32
    Act = mybir.ActivationFunctionType
    Alu = mybir.AluOpType

    nc = bacc.Bacc(None, target_bir_lowering=False)

    q_d = nc.dram_tensor("query", [T, D], f32, kind="ExternalInput")
    k_d = nc.dram_tensor("key", [T, D], f32, kind="ExternalInput")
    v_d = nc.dram_tensor("value", [T, D], f32, kind="ExternalInput")
    m_d = nc.dram_tensor("attention_mask", [1, T], i32, kind="ExternalInput")
    o_d = nc.dram_tensor("out", [T, D], f32, kind="ExternalOutput")

    with tile.TileContext(nc) as tc:
        with (
            tc.tile_pool(name="const", bufs=1) as const_pool,
            tc.tile_pool(name="natq", bufs=N_GROUP) as natq_pool,
            tc.tile_pool(name="natqb", bufs=N_GROUP) as natqb_pool,
            tc.tile_pool(name="natkb", bufs=N_GROUP) as natkb_pool,
            tc.tile_pool(name="natvb", bufs=N_GROUP) as natvb_pool,
            tc.tile_pool(name="qt", bufs=N_GROUP) as qt_pool,
            tc.tile_pool(name="kt", bufs=N_GROUP) as kt_pool,
            tc.tile_pool(name="pt", bufs=4) as pt_pool,
            tc.tile_pool(name="rcp", bufs=8) as rcp_pool,
            tc.tile_pool(name="osb", bufs=8) as osb_pool,
            tc.tile_pool(name="scratch_dram", bufs=1, space="DRAM") as dram_pool,
            tc.tile_pool(name="work_ps", bufs=3, space="PSUM") as work_ps,
            tc.tile_pool(name="o_ps", bufs=SUBS, space="PSUM") as o_ps_pool,
            tc.tile_pool(name="den_ps", bufs=1, space="PSUM") as den_ps_pool,
        ):
            # ---- PE warm-up burst FIRST: ~5us of continuous matmul
            # activity fills a full HAM SHORT window, flipping the PE
            # clock to 2.4 GHz at ~3.5us for the whole kernel. ----
            junk = const_pool.tile([P, D], bf16)
            nc.vector.memset(junk[:], 0.125)
            warm_ps = work_ps.tile([P, D], f32, tag="work")
            n_warm = 12
            for i in range(n_warm):
                nc.tensor.matmul(
                    warm_ps[:],
                    junk[:, :P],
                    junk[:],
                    start=(i == 0),
                    stop=(i == n_warm - 1),
                )

            # ---- sync HWDGE queue: Q groups 0-2 as f32, block-granular
            # (256 KB sequential reads), in need order. Q3 is issued at
            # group-1 time so the Q2 XBAR transpose isn't queued behind
            # its transfers. ----
            natq = [
                natq_pool.tile([P, SUBS, D], f32, tag="natq", name=f"natq{g}")
                for g in range(N_GROUP)
            ]

            def load_q(g):
                for a in range(SUBS):
                    r0 = g * QGROUP + a * P
                    nc.sync.dma_start(natq[g][:, a, :], q_d[r0 : r0 + P, :])

            load_q(0)
            load_q(1)
            load_q(2)

            # ---- gpsimd software-DGE: casting loads (f32 -> bf16 in the
            # DMA pipeline), 1 MB per issue. K0 first (feeds the first
            # XBAR transposes), then V0, then K/V for later groups. ----
            natkb, natvb = [], []
            for g in range(N_GROUP):
                natkb.append(
                    natkb_pool.tile([P, SUBS, D], bf16, tag="natkb", name=f"natkb{g}")
                )
                natvb.append(
                    natvb_pool.tile([P, SUBS, D], bf16, tag="natvb", name=f"natvb{g}")
                )

            def cast_load(dst, src_dram, g):
                rows = src_dram[g * QGROUP : (g + 1) * QGROUP, :]
                nc.gpsimd.dma_start(
                    dst[:], rows.rearrange("(a p) d -> p a d", a=SUBS)
                )

            cast_load(natkb[0], k_d, 0)
            cast_load(natvb[0], v_d, 0)
            for g in range(1, N_GROUP):
                cast_load(natkb[g], k_d, g)
                cast_load(natvb[g], v_d, g)

            # ---- constants ----
            ident = const_pool.tile([P, P], bf16)
            make_identity(nc, ident[:])
            tri = const_pool.tile([P, P], bf16)
            # strictly-lower-triangular NEG (mask S^T where k > q)
            make_lower_triangular(nc, tri[:], val=NEG, diag=False)
            ones = const_pool.tile([P, 8], bf16)
            nc.vector.memset(ones[:], 1.0)

            # key-padding mask -> additive exp bias [128 k_inner, 16 k_chunk]
            mask_i = const_pool.tile([N_KCHUNK, P], i32)
            nc.scalar.dma_start(
                mask_i[:], m_d[0].rearrange("(a b) -> a b", a=N_KCHUNK)
            )
            mb = const_pool.tile([N_KCHUNK, P], bf16)
            nc.vector.tensor_copy(out=mb[:], in_=mask_i[:])
            nc.vector.tensor_scalar(
                mb[:], mb[:], 1.0, 1e30, Alu.subtract, Alu.mult
            )

            # ---- d-major tiles: [128 d_inner, tb/kc, dc, 128] per group ----
            qt_tiles = [
                qt_pool.tile([P, SUBS, N_DSUB, P], bf16, tag="qt", name=f"qt{g}")
                for g in range(N_GROUP)
            ]
            kt_tiles = [
                kt_pool.tile([P, SUBS, N_DSUB, P], bf16, tag="kt", name=f"kt{g}")
                for g in range(N_GROUP)
            ]

            # scalar HWDGE queue: XBAR transposes for K0 (halves, so the
            # first S^T chunk starts ~2us earlier) and K1; it is free of
            # other issue work before the exps begin.
            nc.scalar.dma_start_transpose(kt_tiles[0][:, 0:2], natkb[0][:, 0:2, :])
            nc.scalar.dma_start_transpose(kt_tiles[0][:, 2:4], natkb[0][:, 2:4, :])
            nc.scalar.dma_start_transpose(kt_tiles[1][:], natkb[1][:])

            # Q2/Q3 and K2/K3 XBAR transposes are emitted inside the main
            # loop (program order must place them after their DVE-cast /
            # gpsimd-load producers).
            natqb = [
                natqb_pool.tile([P, SUBS, D], bf16, tag="natqb", name=f"natqb{g}")
                for g in range(N_GROUP)
            ]

            # ---- DVE: Q casts (Q0/Q1 feed PE transposes; Q2/Q3 feed the
            # XBARs above). Emission order = DVE program order. ----
            def cast_q(g):
                for a in range(SUBS):
                    nc.vector.tensor_copy(
                        out=natqb[g][:, a, :], in_=natq[g][:, a, :]
                    )

            cast_q(0)

            # mask bias: transpose [16,128] -> [128,16] on PE via ident
            bias_ps = work_ps.tile([P, N_KCHUNK], bf16, tag="work")
            bias = const_pool.tile([P, N_KCHUNK], bf16)

            def prep_q_pe(g):
                """Transpose Q group g on the PE (4 matmuls + 1 DVE copy
                per t-block) into the same qt layout the XBAR produces."""
                for a in range(SUBS):
                    ps = work_ps.tile([P, D], f32, tag="work")
                    for dc in range(N_DSUB):
                        nc.tensor.matmul(
                            ps[:, dc * P : (dc + 1) * P],
                            natqb[g][:, a, dc * P : (dc + 1) * P],
                            ident[:],
                            start=True,
                            stop=True,
                            skip_group_check=True,
                        )
                    nc.vector.tensor_copy(
                        out=qt_tiles[g][:, a],
                        in_=ps[:].rearrange("p (a b) -> p a b", a=N_DSUB),
                    )

            prep_q_pe(0)
            nc.tensor.transpose(bias_ps[:], mb[:], ident[:N_KCHUNK, :N_KCHUNK])
            nc.vector.tensor_copy(out=bias[:], in_=bias_ps[:])
            cast_q(1)

            # warm-up sink early (keeps the warm-up matmuls live without
            # extending the kernel tail)
            warm_sb = const_pool.tile([P, 1], f32)
            nc.vector.tensor_copy(out=warm_sb[:], in_=warm_ps[:, 0:1])
            warm_dram = dram_pool.tile([P, 1], f32)
            nc.sync.dma_start(warm_dram[:], warm_sb[:])

            # pending chunk whose PV/den matmuls have not been emitted
            # yet: one chunk of slack hides the ScalarE exp latency.
            pending = []

            def epilogue(g, qs):
                rcp = rcp_pool.tile([P, 1], f32, tag="rcp")
                nc.vector.reciprocal(rcp[:], den_ps[:, qs * 8 : qs * 8 + 1])
                osb = osb_pool.tile([P, D], f32, tag="osb")
                if g < 2:
                    nc.scalar.mul(osb[:], o_ps_tiles[qs][:], rcp[:])
                else:
                    nc.vector.tensor_scalar(
                        osb[:], o_ps_tiles[qs][:], rcp[:], None, Alu.mult
                    )
                r0 = g * QGROUP + qs * P
                if g == N_GROUP - 1 and qs % 2 == 1:
                    nc.sync.dma_start(o_d[r0 : r0 + P, :], osb[:])
                else:
                    nc.gpsimd.dma_start(o_d[r0 : r0 + P, :], osb[:])

            def emit_pv(g):
                c, j, pt = pending.pop(0)
                a0 = max(j, 0)
                for qs in range(a0, SUBS):
                    pts = pt[:, (qs - a0) * P : (qs - a0) * P + P]
                    first = c == 0
                    last = c == SUBS * g + qs
                    nc.tensor.matmul(
                        o_ps_tiles[qs][:],
                        pts,
                        natvb[c // SUBS][:, c % SUBS, :],
                        start=first,
                        stop=last,
                    )
                    # All four qs columns share one PSUM bank; start=True
                    # clears has_written bank-wide, so only the very first
                    # den matmul of the group may set it.
                    nc.tensor.matmul(
                        den_ps[:, qs * 8 : qs * 8 + 8],
                        pts,
                        ones[:],
                        start=(first and qs == a0),
                        stop=last,
                        skip_group_check=True,
                    )
                    if last:
                        epilogue(g, qs)

            for g in range(N_GROUP):
                o_ps_tiles = [
                    o_ps_pool.tile([P, D], f32, tag="o", name=f"o_{g}_{i}")
                    for i in range(SUBS)
                ]
                den_ps = den_ps_pool.tile([P, SUBS * 8], f32, tag="den")

                n_chunks = SUBS * (g + 1)
                # Late prep, one group ahead of need. Sync-queue order:
                # [Q0-2 loads, warm-sink, xQ2, Q3 loads, xK2, xK3, xQ3]
                # keeps every transpose ahead of its first-use deadline.
                if g == 1:
                    cast_q(2)
                    nc.sync.dma_start_transpose(qt_tiles[2][:], natqb[2][:])
                    load_q(3)
                if g == 2:
                    cast_q(3)
                    nc.sync.dma_start_transpose(kt_tiles[2][:], natkb[2][:])
                    nc.sync.dma_start_transpose(kt_tiles[3][:], natkb[3][:])
                    nc.sync.dma_start_transpose(qt_tiles[3][:], natqb[3][:])
                for c in range(n_chunks):
                    j = c - SUBS * g  # >= 0 on the diagonal band
                    a0 = max(j, 0)
                    width = QGROUP - P * a0
                    st = work_ps.tile([P, D], f32, tag="work")
                    for dc in range(N_DSUB):
                        nc.tensor.matmul(
                            st[:, :width],
                            kt_tiles[c // SUBS][:, c % SUBS, dc, :],
                            qt_tiles[g][:, a0:, dc, :],
                            start=(dc == 0),
                            stop=(dc == N_DSUB - 1),
                        )
                    if j >= 0:
                        # causal mask on the diagonal 128x128 block:
                        # ident.T @ tri == tri added into the accumulation
                        nc.tensor.matmul(
                            st[:, :P],
                            ident[:],
                            tri[:],
                            start=False,
                            stop=True,
                            skip_group_check=True,
                        )
                    pt = pt_pool.tile([P, QGROUP], bf16, tag="pt")
                    nc.scalar.activation(
                        out=pt[:, :width],
                        in_=st[:, :width],
                        func=Act.Exp,
                        bias=bias[:, c : c + 1],
                        scale=SCALE,
                    )
                    if len(pending) >= 1:
                        emit_pv(g)
                    pending.append((c, j, pt))
                    # Q1 is transposed on the PE right as its DVE casts
                    # land, before group 0's final PV flush.
                    if g == 0 and c == n_chunks - 1:
                        prep_q_pe(1)
                while pending:
                    emit_pv(g)

    nc.finalize()
    return nc


def _get_nc():
    if "nc" not in _CACHE:
        _CACHE["nc"] = _build()
    return _CACHE["nc"]


def kernel(**inputs):
    from concourse.bass_utils import run_bass_kernel_spmd

    q = np.ascontiguousarray(np.asarray(inputs["query"], dtype=np.float32))
    k = np.ascontiguousarray(np.asarray(inputs["key"], dtype=np.float32))
    v = np.ascontiguousarray(np.asarray(inputs["value"], dtype=np.float32))
    m = np.ascontiguousarray(
        np.asarray(inputs["attention_mask"], dtype=np.int32)
    )

    nc = _get_nc()
    in_maps = [
        {
            "query": q[i],
            "key": k[i],
            "value": v[i],
            "attention_mask": m[i].reshape(1, T),
        }
        for i in range(B)
    ]
    trace = os.environ.get("BASS_KERNEL_TRACE", "0") == "1"
    res = run_bass_kernel_spmd(
        nc, in_maps, core_ids=list(range(B)), trace=trace
    )
    _CACHE["last_result"] = res
    out = np.stack([r["out"] for r in res.results]).astype(np.float32)
    return out


# revision 9
# speedup vs baseline: 1.1097x; 1.1097x over previous
"""Causal scaled-dot-product attention on 8 TRN2 NeuronCores.

Problem: B=8, Tq=Tk=2048, D=512, f32, causal + key-padding mask.
Sharding: batch-parallel — core i handles batch element i; no collectives.

Per-core algorithm (one batch element, all on one NeuronCore):
  * S^T[k, q] = sum_d KT_chunk^T @ QT per 128-wide k chunk (PE bf16,
    4 accumulating matmuls); the strictly-lower-triangular -1e30 causal
    tile is folded into the same PSUM accumulation as a 5th matmul
    (ident.T @ tri adds tri elementwise, 128 cols ~ 55 ns).
  * P^T = exp(S^T * 1/sqrt(D) + key_bias[k]) on ScalarE (key padding
    mask folds into the per-partition activation bias).
  * out[q,:] += P^T_chunk^T @ V_chunk and denom[q] += P^T_chunk^T @ ones
    (PE, stationary reuse); PV emitted one chunk behind S^T to hide exp.
  * Per q-block epilogue: out *= 1/denom (ScalarE for groups 0-2, DVE
    for the last group so the tail is not serialized behind exps).

Data movement:
  * 10 warm-up matmuls on memset data run first; together with the
    group-0 transpose matmuls they give ~5us of near-continuous PE
    activity, flipping the HAM clock-gate to 8/8 (2.4 GHz) at ~3.5us
    instead of ~30us.
  * DMA count is kept low (~30) and transfers large (mostly 1 MB):
    the DMA flow-control semaphore ring is shared across queues, so
    many small DMAs serialize cross-queue and collapse bandwidth.
  * Q0/K0 stream as four 256 KB f32 blocks each (sync/scalar HWDGE) so
    prep can start on partial data; Q1-3 are single 1 MB f32 loads on
    sync. K1-3 and V0-3 are gpsimd software-DGE casting loads (f32
    DRAM -> bf16 SBUF, converted inside the DMA engine); V tiles are
    used by the PV matmuls with no further processing.
  * d-major (transposed) layouts: Q0/Q1/K0 via PE transpose matmuls
    (they are needed before a cast+XBAR chain could deliver them, and
    the PE is idle that early); K1-3 and Q2/Q3 via XBAR transpose-DMAs
    (dma_start_transpose, bf16 SBUF->SBUF, one 512 KB call per group)
    on the sync queue. XBAR destinations must be contiguous tiles
    (strided dsts corrupt on HW), hence the [128, tb, dc, 128] group
    layout with 2-free-dim moving operands in the S^T matmuls.
  * Stores: groups 0-2 are single 1 MB stores on the gpsimd queue
    (idle after the casting loads); the last group stores per-q-block,
    split gpsimd/sync, to shorten the tail.

No max-subtraction: post-scale scores are ~N(0,1), so exp is safe in
f32 and softmax is shift-invariant.
"""

import os

import numpy as np

B = 8
T = 2048
D = 512
P = 128
NEG = -1e30
SCALE = 1.0 / float(np.sqrt(np.float32(D)))

N_DSUB = D // P  # 4 d-chunks of 128
N_KCHUNK = T // P  # 16 k-chunks of 128
QGROUP = 512
N_GROUP = T // QGROUP  # 4 q-groups
SUBS = QGROUP // P  # 4 q/k-subblocks of 128 per group

_CACHE = {}


def _build():
    import concourse.bass as bass  # noqa: F401
    import concourse.mybir as mybir
    import concourse.tile as tile
    from concourse import bacc
    from concourse.masks import make_identity, make_lower_triangular

    f32 = mybir.dt.float32
    bf16 = mybir.dt.bfloat16
    i32 = mybir.dt.int32
    Act = mybir.ActivationFunctionType
    Alu = mybir.AluOpType

    nc = bacc.Bacc(None, target_bir_lowering=False)

    q_d = nc.dram_tensor("query", [T, D], f32, kind="ExternalInput")
    k_d = nc.dram_tensor("key", [T, D], f32, kind="ExternalInput")
    v_d = nc.dram_tensor("value", [T, D], f32, kind="ExternalInput")
    m_d = nc.dram_tensor("attention_mask", [1, T], i32, kind="ExternalInput")
    o_d = nc.dram_tensor("out", [T, D], f32, kind="ExternalOutput")

    def grouped(dram, g):
        return dram[g * QGROUP : (g + 1) * QGROUP, :].rearrange(
            "(a p) d -> p a d", a=SUBS
        )

    with tile.TileContext(nc) as tc:
        with (
            tc.tile_pool(name="const", bufs=1) as const_pool,
            tc.tile_pool(name="natq", bufs=N_GROUP) as natq_pool,
            tc.tile_pool(name="natk0", bufs=1) as natk0_pool,
            tc.tile_pool(name="natqb", bufs=N_GROUP) as natqb_pool,
            tc.tile_pool(name="natkb", bufs=N_GROUP) as natkb_pool,
            tc.tile_pool(name="natvb", bufs=N_GROUP) as natvb_pool,
            tc.tile_pool(name="qt", bufs=N_GROUP) as qt_pool,
            tc.tile_pool(name="kt", bufs=N_GROUP) as kt_pool,
            tc.tile_pool(name="pt", bufs=4) as pt_pool,
            tc.tile_pool(name="rcp", bufs=8) as rcp_pool,
            tc.tile_pool(name="osbg", bufs=3) as osbg_pool,
            tc.tile_pool(name="osb", bufs=4) as osb_pool,
            tc.tile_pool(name="scratch_dram", bufs=1, space="DRAM") as dram_pool,
            tc.tile_pool(name="work_ps", bufs=3, space="PSUM") as work_ps,
            tc.tile_pool(name="o_ps", bufs=SUBS, space="PSUM") as o_ps_pool,
            tc.tile_pool(name="den_ps", bufs=1, space="PSUM") as den_ps_pool,
        ):
            # ---- PE warm-up burst FIRST (~4.3us at the cold clock);
            # group-0 transpose matmuls follow seamlessly, so the HAM
            # SHORT window fills and the clock flips to 2.4 GHz early.
            junk = const_pool.tile([P, D], bf16)
            nc.vector.memset(junk[:], 0.125)
            warm_ps = work_ps.tile([P, D], f32, tag="work")
            n_warm = 10
            for i in range(n_warm):
                nc.tensor.matmul(
                    warm_ps[:],
                    junk[:, :P],
                    junk[:],
                    start=(i == 0),
                    stop=(i == n_warm - 1),
                )

            # ---- input tiles ----
            natq = [
                natq_pool.tile([P, SUBS, D], f32, tag="natq", name=f"natq{g}")
                for g in range(N_GROUP)
            ]
            natk0 = natk0_pool.tile([P, SUBS, D], f32, tag="natk0")
            natqb = [
                natqb_pool.tile([P, SUBS, D], bf16, tag="natqb", name=f"natqb{g}")
                for g in range(N_GROUP)
            ]
            natkb = [
                natkb_pool.tile([P, SUBS, D], bf16, tag="natkb", name=f"natkb{g}")
                for g in range(N_GROUP)
            ]
            natvb = [
                natvb_pool.tile([P, SUBS, D], bf16, tag="natvb", name=f"natvb{g}")
                for g in range(N_GROUP)
            ]

            # sync: Q0 block-granular (prep starts on partial data),
            # then Q1/Q2 as single 1 MB loads. Q3 is issued at group-1
            # time so the XBAR transposes are not queued behind it.
            for a in range(SUBS):
                nc.sync.dma_start(natq[0][:, a, :], q_d[a * P : (a + 1) * P, :])
            nc.sync.dma_start(natq[1][:], grouped(q_d, 1))
            nc.sync.dma_start(natq[2][:], grouped(q_d, 2))

            # scalar: mask + K0 block-granular; free for exps afterward.
            mask_i = const_pool.tile([N_KCHUNK, P], i32)
            nc.scalar.dma_start(
                mask_i[:], m_d[0].rearrange("(a b) -> a b", a=N_KCHUNK)
            )
            for a in range(SUBS):
                nc.scalar.dma_start(natk0[:, a, :], k_d[a * P : (a + 1) * P, :])

            # gpsimd software-DGE casting loads (f32 -> bf16), 1 MB each,
            # in need order. V tiles are consumed directly by PV matmuls.
            nc.gpsimd.dma_start(natvb[0][:], grouped(v_d, 0))
            for g in range(1, N_GROUP):
                nc.gpsimd.dma_start(natkb[g][:], grouped(k_d, g))
                nc.gpsimd.dma_start(natvb[g][:], grouped(v_d, g))

            # ---- constants ----
            ident = const_pool.tile([P, P], bf16)
            make_identity(nc, ident[:])
            tri = const_pool.tile([P, P], bf16)
            # strictly-lower-triangular NEG (mask S^T where k > q)
            make_lower_triangular(nc, tri[:], val=NEG, diag=False)
            ones = const_pool.tile([P, 8], bf16)
            nc.vector.memset(ones[:], 1.0)

            mb = const_pool.tile([N_KCHUNK, P], bf16)
            nc.vector.tensor_copy(out=mb[:], in_=mask_i[:])
            nc.vector.tensor_scalar(
                mb[:], mb[:], 1.0, 1e30, Alu.subtract, Alu.mult
            )

            # ---- d-major tiles: [128 d_inner, tb/kc, dc, 128] per group ----
            qt_tiles = [
                qt_pool.tile([P, SUBS, N_DSUB, P], bf16, tag="qt", name=f"qt{g}")
                for g in range(N_GROUP)
            ]
            kt_tiles = [
                kt_pool.tile([P, SUBS, N_DSUB, P], bf16, tag="kt", name=f"kt{g}")
                for g in range(N_GROUP)
            ]

            # ---- group-0 prep on the PE (interleaved per t-block):
            # cast (DVE) -> 4 transpose matmuls -> PSUM->SBUF copy
            # (K0 copies on ScalarE, Q copies on DVE). ----
            def cast_q(g):
                for a in range(SUBS):
                    nc.vector.tensor_copy(
                        out=natqb[g][:, a, :], in_=natq[g][:, a, :]
                    )

            def prep_pe(natb, dst, a, copy_eng):
                ps = work_ps.tile([P, D], f32, tag="work")
                for dc in range(N_DSUB):
                    nc.tensor.matmul(
                        ps[:, dc * P : (dc + 1) * P],
                        natb[:, a, dc * P : (dc + 1) * P],
                        ident[:],
                        start=True,
                        stop=True,
                        skip_group_check=True,
                    )
                src = ps[:].rearrange("p (a b) -> p a b", a=N_DSUB)
                if copy_eng == "scalar":
                    nc.scalar.copy(dst[:, a], src)
                else:
                    nc.vector.tensor_copy(out=dst[:, a], in_=src)

            for a in range(SUBS):
                nc.vector.tensor_copy(
                    out=natkb[0][:, a, :], in_=natk0[:, a, :]
                )
                nc.vector.tensor_copy(
                    out=natqb[0][:, a, :], in_=natq[0][:, a, :]
                )
                prep_pe(natkb[0], kt_tiles[0], a, "scalar")
                prep_pe(natqb[0], qt_tiles[0], a, "vector")

            # mask bias: transpose [16,128] -> [128,16] on the PE
            bias_ps = work_ps.tile([P, N_KCHUNK], bf16, tag="work")
            nc.tensor.transpose(bias_ps[:], mb[:], ident[:N_KCHUNK, :N_KCHUNK])
            bias = const_pool.tile([P, N_KCHUNK], bf16)
            nc.vector.tensor_copy(out=bias[:], in_=bias_ps[:])

            cast_q(1)

            # warm-up sink early (keeps the warm-up matmuls live without
            # extending the kernel tail)
            warm_sb = const_pool.tile([P, 1], f32)
            nc.vector.tensor_copy(out=warm_sb[:], in_=warm_ps[:, 0:1])
            warm_dram = dram_pool.tile([P, 1], f32)
            nc.sync.dma_start(warm_dram[:], warm_sb[:])

            # pending chunk whose PV/den matmuls have not been emitted
            # yet: one chunk of slack hides the ScalarE exp latency.
            pending = []

            def epilogue(g, qs):
                rcp = rcp_pool.tile([P, 1], f32, tag="rcp")
                nc.vector.reciprocal(rcp[:], den_ps[:, qs * 8 : qs * 8 + 1])
                if g < N_GROUP - 1:
                    osb = osbg_tile[:, qs, :]
                    nc.scalar.mul(osb, o_ps_tiles[qs][:], rcp[:])
                    if qs == SUBS - 1:
                        nc.gpsimd.dma_start(grouped(o_d, g), osbg_tile[:])
                else:
                    osb = osb_pool.tile([P, D], f32, tag="osb")
                    nc.vector.tensor_scalar(
                        osb[:], o_ps_tiles[qs][:], rcp[:], None, Alu.mult
                    )
                    r0 = g * QGROUP + qs * P
                    eng = nc.sync if qs % 2 == 1 else nc.gpsimd
                    eng.dma_start(o_d[r0 : r0 + P, :], osb[:])

            def emit_pv(g):
                c, j, pt = pending.pop(0)
                a0 = max(j, 0)
                for qs in range(a0, SUBS):
                    pts = pt[:, (qs - a0) * P : (qs - a0) * P + P]
                    first = c == 0
                    last = c == SUBS * g + qs
                    nc.tensor.matmul(
                        o_ps_tiles[qs][:],
                        pts,
                        natvb[c // SUBS][:, c % SUBS, :],
                        start=first,
                        stop=last,
                    )
                    # All four qs columns share one PSUM bank; start=True
                    # clears has_written bank-wide, so only the very first
                    # den matmul of the group may set it.
                    nc.tensor.matmul(
                        den_ps[:, qs * 8 : qs * 8 + 8],
                        pts,
                        ones[:],
                        start=(first and qs == a0),
                        stop=last,
                        skip_group_check=True,
                    )
                    if last:
                        epilogue(g, qs)

            for g in range(N_GROUP):
                o_ps_tiles = [
                    o_ps_pool.tile([P, D], f32, tag="o", name=f"o_{g}_{i}")
                    for i in range(SUBS)
                ]
                den_ps = den_ps_pool.tile([P, SUBS * 8], f32, tag="den")
                if g < N_GROUP - 1:
                    osbg_tile = osbg_pool.tile(
                        [P, SUBS, D], f32, tag="osbg", name=f"osbg{g}"
                    )

                # Late prep, one group ahead of need. Sync-queue order:
                # [Q0 blocks, Q1, Q2, warm-sink, xK1, xQ2, Q3, xK2, xK3,
                # xQ3, tail stores].
                if g == 1:
                    nc.sync.dma_start_transpose(kt_tiles[1][:], natkb[1][:])
                    cast_q(2)
                    nc.sync.dma_start_transpose(qt_tiles[2][:], natqb[2][:])
                    nc.sync.dma_start(natq[3][:], grouped(q_d, 3))
                if g == 2:
                    nc.sync.dma_start_transpose(kt_tiles[2][:], natkb[2][:])
                    nc.sync.dma_start_transpose(kt_tiles[3][:], natkb[3][:])
                    cast_q(3)
                    nc.sync.dma_start_transpose(qt_tiles[3][:], natqb[3][:])

                n_chunks = SUBS * (g + 1)
                for c in range(n_chunks):
                    j = c - SUBS * g  # >= 0 on the diagonal band
                    a0 = max(j, 0)
                    width = QGROUP - P * a0
                    st = work_ps.tile([P, D], f32, tag="work")
                    for dc in range(N_DSUB):
                        nc.tensor.matmul(
                            st[:, :width],
                            kt_tiles[c // SUBS][:, c % SUBS, dc, :],
                            qt_tiles[g][:, a0:, dc, :],
                            start=(dc == 0),
                            stop=(dc == N_DSUB - 1),
                        )
                    if j >= 0:
                        # causal mask on the diagonal 128x128 block:
                        # ident.T @ tri == tri added into the accumulation
                        nc.tensor.matmul(
                            st[:, :P],
                            ident[:],
                            tri[:],
                            start=False,
                            stop=True,
                            skip_group_check=True,
                        )
                    pt = pt_pool.tile([P, QGROUP], bf16, tag="pt")
                    nc.scalar.activation(
                        out=pt[:, :width],
                        in_=st[:, :width],
                        func=Act.Exp,
                        bias=bias[:, c : c + 1],
                        scale=SCALE,
                    )
                    if len(pending) >= 1:
                        emit_pv(g)
                    pending.append((c, j, pt))
                    # Q1 is transposed on the PE right as its DVE casts
                    # land, before group 0's final PV flush.
                    if g == 0 and c == n_chunks - 1:
                        for a in range(SUBS):
                            prep_pe(natqb[1], qt_tiles[1], a, "vector")
                while pending:
                    emit_pv(g)

    nc.finalize()
    return nc


def _get_nc():
    if "nc" not in _CACHE:
        _CACHE["nc"] = _build()
    return _CACHE["nc"]


def kernel(**inputs):
    from concourse.bass_utils import run_bass_kernel_spmd

    q = np.ascontiguousarray(np.asarray(inputs["query"], dtype=np.float32))
    k = np.ascontiguousarray(np.asarray(inputs["key"], dtype=np.float32))
    v = np.ascontiguousarray(np.asarray(inputs["value"], dtype=np.float32))
    m = np.ascontiguousarray(
        np.asarray(inputs["attention_mask"], dtype=np.int32)
    )

    nc = _get_nc()
    in_maps = [
        {
            "query": q[i],
            "key": k[i],
            "value": v[i],
            "attention_mask": m[i].reshape(1, T),
        }
        for i in range(B)
    ]
    trace = os.environ.get("BASS_KERNEL_TRACE", "0") == "1"
    res = run_bass_kernel_spmd(
        nc, in_maps, core_ids=list(range(B)), trace=trace
    )
    _CACHE["last_result"] = res
    out = np.stack([r["out"] for r in res.results]).astype(np.float32)
    return out
